# revision 28
# baseline (speedup 1.0000x reference)
"""Trainium2 Bass kernel for nn_AttentionHead_51178830299302.

Single attention head: B=8, S=2048, E=1024, H=64, fp32 I/O, decoder
(causal) masking plus a pad-pad coupling term (padded queries attend
bidirectionally to padded keys).

Strategy:
  * Data-parallel over batch: one batch element per NeuronCore (8 cores).
  * Host-side, each sequence is stably partitioned into [valid | pad]
    positions (order preserved).  The masked softmax then decomposes
    exactly into two independent attention problems:
      - valid x valid with plain causal masking,
      - pad x pad with full bidirectional softmax (no mask),
    which skips ~60% of the S x S exp/matmul work vs. the dense mask.
  * bf16 matmul pipeline (fp32 PSUM accumulation), exp on ScalarE
    straight from PSUM.
  * HBM bandwidth (~360 GB/s, shared) binds the head of the kernel and
    a single DMA ring tops out at ~260 GB/s, so the bulk hidden-state
    stream is split across the scalar + gpsimd rings (half-slices
    each), while the SYNC ring stays empty for the latency-critical
    partition-bounce / V-transpose / output hops (rings drain FIFO, so
    a small hop queued behind bulk would stall until the bulk drains).
  * k projects with NO bias on device: softmax is invariant to the
    per-query constant <bk, q> it would add, and slot-padded keys
    (zeroed hidden state) then score exactly 0 -> exp 1, contributing
    nothing to the numerator (their v rows are 0) and a per-batch
    constant to the pad-part denominators that the host divide
    subtracts.  No kill row -> score contraction depth K=64, so score
    matmuls for two key chunks run CONCURRENTLY as PE row-tiles
    (rows 0:64 vs 64:128) on mirrored q/k copies.
  * q/k land in both partition halves: direct evacuation into one half,
    one 64-row SBUF->SBUF bounce into the other.
  * V is moved to natural [seq, head] layout with one XBAR transpose
    DMA per 512-slice; attention output is produced transposed
    [H+1, seq] with the softmax row-sums riding as an appended ones
    column of V; final divide+transpose happens on host.
  * Causal masking inside diagonal 128-blocks is one bf16 multiply
    with a constant 0/1 tril tile.

kernel(**inputs) takes the FULL unsharded fp32 inputs and returns the
FULL [8, 2048, 64] fp32 output.
"""

import numpy as np
import ml_dtypes

B, S, E, H = 8, 2048, 1024, 64
P = 128
BF = ml_dtypes.bfloat16

_NC_CACHE: dict = {}


def _patch_tile_drain():
    """The stock TileContext exit hangs every global-clock wait on a single
    Drain instruction; this container's walrus caps sync waits at 1 per
    instruction.  Split the waits across single-wait nops, and drop the
    second (post-semclear) all-engine barrier — engines halt right after,
    and NEFF re-execution only starts once every engine has halted."""
    import concourse.tile as tile
    import concourse.mybir as mybir
    from bass_rust import ScopedClock

    if getattr(tile.TileContext, "_drain_waits_split", False):
        return

    def _drain_and_barrier(self, tick_clock, wait_clock):
        nc = self.nc
        carrier = nc.sync.nop(nofuse=True)
        wait_clock.add_sem_waits(
            carrier.ins, ScopedClock({None: tick_clock.global_clock})
        )
        si = carrier.ins.sync_info
        waits = list(si.on_wait) if si and si.on_wait else []
        if len(waits) > 1:
            si.on_wait = waits[:1]
            for w in waits[1:]:
                n = nc.sync.nop(nofuse=True)
                nsi = n.ins.sync_info
                if nsi is None:
                    n.ins.sync_info = mybir.SyncInfo(on_wait=[w], on_update=[])
                else:
                    nsi.on_wait = [w]
        nc.sync.drain()
        nc.all_engine_barrier(sem_only=True)
        popped = nc._tile_sem_poison_stack.pop()
        assert popped is self._sem_poison
        nc.clear_and_free_semaphores(list(self.sems.allocated().values()))

    tile.TileContext._drain_and_barrier = _drain_and_barrier
    tile.TileContext._drain_waits_split = True


def _patch_sync_wait_split():
    """This container's walrus codegen rejects instructions carrying more
    than one sync wait.  Post-process the serialized BIR: hoist excess
    waits onto injected NoOps on the same engine, just before the
    instruction (the sequencer executes them in order, so semantics are
    preserved)."""
    import json
    import concourse.bass as bass

    if getattr(bass.Bass, "_sync_wait_split", False):
        return
    orig = bass.Bass.to_json_bytes

    def to_json_bytes(self) -> bytes:
        j = json.loads(orig(self))
        ctr = [0]

        def fix_block(blk):
            insts = blk.get("instructions")
            if not isinstance(insts, list):
                return
            out = []
            for inst in insts:
                si = inst.get("sync_info")
                ow = (si or {}).get("on_wait") or []
                if len(ow) > 1:
                    si["on_wait"] = ow[-1:]
                    for w in ow[:-1]:
                        ctr[0] += 1
                        out.append(
                            {
                                "debug": inst.get("debug", 0),
                                "engine": inst["engine"],
                                "ins": [],
                                "name": f"I-wsplit-{ctr[0]}",
                                "opcode": "NoOp",
                                "outs": [],
                                "sync_info": {"on_wait": [w], "on_update": []},
                            }
                        )
                out.append(inst)
            blk["instructions"] = out

        def rec(o):
            if isinstance(o, dict):
                if "instructions" in o:
                    fix_block(o)
                for v in o.values():
                    rec(v)
            elif isinstance(o, list):
                for v in o:
                    rec(v)

        rec(j)
        return json.dumps(j).encode()

    bass.Bass.to_json_bytes = to_json_bytes
    bass.Bass._sync_wait_split = True


def build_nc(SV: int, SP: int):
    """Build the SPMD per-core Bass program.

    Per-core DRAM tensors:
      hsT  [P, NSF, 8, 512]  bf16  full 512-col slices of the sorted,
                                   transposed hidden state
      hsTt [P, 8, WT]        bf16  the final partial slice (WT cols)
      wqk  [P, 8, P]   bf16  [Wq/sqrt(H) | Wk]
      wv   [P, 8, H]   bf16
      bqk  [P, 1]      f32   [bq/sqrt(H) ; 0]  (k takes no device bias)
      c01  [P, 1024]   bf16  tril keep-mask: c01[j, 512+y] = (j <= y)
      outT [65, SVP]   f32   rows 0..63 unnormalized output^T, row 64
                             softmax denominators (host divides)
    """
    import concourse.bass as bass
    import concourse.mybir as mybir
    import concourse.tile as tile
    from contextlib import ExitStack

    _patch_tile_drain()
    _patch_sync_wait_split()
    bf, f32 = mybir.dt.bfloat16, mybir.dt.float32
    Exp = mybir.ActivationFunctionType.Exp

    SVP = SV + SP
    NKC_V, NKC_P = SV // P, SP // P
    NT = SVP // P

    nc = bass.Bass("TRN2", target_bir_lowering=False, debug=False)
    NSL = (SVP + 511) // 512  # number of 512-col projection slices
    WT = SVP - (NSL - 1) * 512  # width of the final slice
    NSF = NSL - 1  # full slices
    hsT_d = nc.dram_tensor("hsT", [P, NSF, 8, 512], bf, kind="ExternalInput").ap()
    hsTt_d = nc.dram_tensor("hsTt", [P, 8, WT], bf, kind="ExternalInput").ap()
    wqk_d = nc.dram_tensor("wqk", [P, 8, P], bf, kind="ExternalInput").ap()
    wv_d = nc.dram_tensor("wv", [P, 8, H], bf, kind="ExternalInput").ap()
    bqk_d = nc.dram_tensor("bqk", [P, 1], f32, kind="ExternalInput").ap()
    c01_d = nc.dram_tensor("c01", [P, 1024], bf, kind="ExternalInput").ap()
    outT_d = nc.dram_tensor("outT", [H + 1, SVP], f32, kind="ExternalOutput").ap()

    with tile.TileContext(nc) as tc, ExitStack() as ctx:
        singles = ctx.enter_context(tc.tile_pool(name="singles", bufs=1))

        # PE warm-up source tile: memset first so the warm-up matmul
        # stream starts as soon as the engines come up.
        wz = singles.tile([P, 512], bf)
        nc.vector.memset(wz[:], 0.0)
        ones_t = singles.tile([1, P], bf)
        nc.vector.memset(ones_t[:], 1.0)

        wqk_s = singles.tile([P, 8, P], bf)
        wv_s = singles.tile([P, 8, H], bf)
        bqk_s = singles.tile([P, 1], f32)
        c01_s = singles.tile([P, 1024], bf)

        # q/k in BOTH partition halves (row-tile score pairing):
        # rows 0:64 = "lo" copy, rows 64:128 = "hi" copy.
        # Direct evacuation: q -> lo (PSUM rows 0:64), k -> hi (rows
        # 64:128); the other half arrives via one SBUF->SBUF bounce.
        qT = singles.tile([P, SVP], bf)
        kT = singles.tile([P, SVP], bf)

        # V in natural [seq-part, head] layout with an appended ones
        # column (row-sums of the attention weights ride along in the
        # AV matmul as output row H).
        vS = singles.tile([P, NT, H + 1], bf)
        nc.vector.memset(vS[:, :, H : H + 1], 1.0)
        vT = singles.tile([P, SVP], bf)
        # XBAR transpose needs a contiguous destination on HW; stage here,
        # then strided-copy into vS (which carries the ones column).
        vN = singles.tile([P, NT, H], bf)

        # One HWDGE ring tops out at ~260 GB/s; only two rings together
        # reach the ~360 GB/s HBM ceiling.  And HWDGE rings drain FIFO,
        # so latency-critical hops must not share a ring with bulk.
        # Split: scalar + gpsimd carry the bulk halves (A = E-chunks
        # 0:4, B = 4:8), the SYNC ring stays EMPTY for the bounce /
        # transpose / output hops.  Constants interleave into the
        # scalar stream where they are first needed.
        # All pieces use 4 KB-per-partition descriptors: the SDMA engines
        # round-robin queues at packet granularity, so unequal packet
        # sizes skew the bandwidth split.
        hsT = singles.tile([P, NSF, 8, 512], bf)
        hsTt = singles.tile([P, 8, WT], bf)
        nc.scalar.dma_start(wqk_s[:], wqk_d)
        nc.scalar.dma_start(hsT[:, 0, 0:4, :], hsT_d[:, 0, 0:4, :])
        nc.scalar.dma_start(bqk_s[:], bqk_d)
        nc.scalar.dma_start(c01_s[:], c01_d)
        nc.scalar.dma_start(hsT[:, 1, 0:4, :], hsT_d[:, 1, 0:4, :])
        nc.scalar.dma_start(wv_s[:], wv_d)
        for si in range(2, NSF):
            nc.scalar.dma_start(hsT[:, si, 0:4, :], hsT_d[:, si, 0:4, :])
        for si in range(NSF):
            nc.gpsimd.dma_start(hsT[:, si, 4:8, :], hsT_d[:, si, 4:8, :])
        nc.gpsimd.dma_start(hsTt[:, 0:4, :], hsTt_d[:, 0:4, :])
        nc.gpsimd.dma_start(hsTt[:, 4:8, :], hsTt_d[:, 4:8, :])

        def hs_chunk(si, c, w):
            if si < NSF:
                return hsT[:, si, c, :w]
            return hsTt[:, c, :w]

        # ------- interleaved projections + attention -------
        # Emission order interleaves projection slices with attention
        # q-blocks whose inputs are already covered, so the PE stream has
        # no phase barrier and HAM stays warm.
        with tc.tile_pool(name="pp", bufs=1, space="PSUM") as pp, \
             tc.tile_pool(name="acc", bufs=2, space="PSUM") as acc, \
             tc.tile_pool(name="spsum", bufs=2, space="PSUM") as spsum, \
             tc.tile_pool(name="wpool", bufs=10) as wpool, \
             tc.tile_pool(name="opool", bufs=3) as opool, \
             tc.tile_pool(name="warmp", bufs=1, space="PSUM") as warmp:

            # 18 cold N=256 matmuls = ~3.8us of sustained PE activity:
            # enough to trip HAM to K=8/8 (its window is 3.4us) right as
            # the first hsT slice lands.  Every later PE gap is shorter
            # than the 3.4us MID window, so the PE then STAYS at 2.4GHz
            # for the whole kernel.  They borrow a pp buffer: the first
            # qk slice is behind them in the PE FIFO anyway.
            warm_ps = warmp.tile([P, 256], f32)
            for _ in range(18):
                nc.tensor.matmul(
                    warm_ps[:], lhsT=wz[:, 0:P], rhs=wz[:, 0:256],
                    start=True, stop=True,
                )

            # Contraction-chunk order: the B-half (gpsimd ring) lands
            # slightly before the A-half, so start with chunks 4:8.
            CORD = [4, 5, 6, 7, 0, 1, 2, 3]

            def emit_qk_slice(sb):
                si = sb // 512
                w = min(512, SVP - sb)
                ps = pp.tile([P, 512], f32)
                for ci, c in enumerate(CORD):
                    nc.tensor.matmul(
                        ps[:, :w],
                        lhsT=wqk_s[:, c, :],
                        rhs=hs_chunk(si, c, w),
                        start=(ci == 0),
                        stop=(ci == 7),
                    )
                # q -> partitions 0:64 (bias add), k -> partitions 64:128
                # (bias row is 0 there; the add is just the bf16 cast).
                nc.vector.tensor_scalar_add(
                    qT[0:64, sb : sb + w], ps[0:64, :w], bqk_s[0:64, 0:1]
                )
                nc.vector.tensor_scalar_add(
                    kT[64:128, sb : sb + w], ps[64:128, :w], bqk_s[64:128, 0:1]
                )
                # mirror each into the other partition half (engines
                # cannot shift partitions; DMA can).  sync = the empty
                # latency ring: lands ~0.4us after the evacuation.
                nc.sync.dma_start(kT[0:64, sb : sb + w], kT[64:128, sb : sb + w])
                nc.sync.dma_start(qT[64:128, sb : sb + w], qT[0:64, sb : sb + w])

            def _v_finish(s, rows, pvd):
                w = min(512, SVP - s)
                cp = nc.vector.tensor_copy(vT[rows[0] : rows[1], s : s + w],
                                           pvd[rows[0] : rows[1], :w])
                ta, tb = s // P, (s + w) // P
                nc.sync.dma_start_transpose(
                    vN[:, ta:tb, :], vT[rows[0] : rows[1], s : s + w]
                )
                nc.vector.tensor_copy(vS[:, ta:tb, 0:H], vN[:, ta:tb, :])
                return cp

            def emit_v_pair(sA, sB):
                # V projection for two 512-slices concurrently via PE
                # column tiling: slice A in array cols 0:64 -> PSUM rows
                # 0:64, slice B in cols 64:128 -> PSUM rows 64:128.  A
                # K=1 zero matmul opens the accumulation group for the
                # whole bank (per-chain start=True would clear the
                # sibling chain's has_written bits).
                pvd = acc.tile([P, 512], f32, tag="acc", name="pvd")
                wA = min(512, SVP - sA)
                if sB is None:
                    for c in range(8):
                        nc.tensor.matmul(
                            pvd[0:H, :wA],
                            lhsT=wv_s[:, c, :],
                            rhs=hs_chunk(sA // 512, c, wA),
                            start=(c == 0),
                            stop=(c == 7),
                        )
                    _v_finish(sA, (0, H), pvd)
                    return
                wB = min(512, SVP - sB)
                nc.tensor.matmul(
                    pvd[:, 0:512], lhsT=ones_t[:], rhs=wz[0:1, 0:512],
                    start=True, stop=True, skip_group_check=True,
                )
                for c in range(8):
                    nc.tensor.matmul(
                        pvd[0:H, :wA],
                        lhsT=wv_s[:, c, :],
                        rhs=hs_chunk(sA // 512, c, wA),
                        start=False, stop=(c == 7), tile_position=(0, 0),
                        skip_group_check=True,
                    )
                    nc.tensor.matmul(
                        pvd[H:P, :wB],
                        lhsT=wv_s[:, c, :],
                        rhs=hs_chunk(sB // 512, c, wB),
                        start=False, stop=(c == 7), tile_position=(0, H),
                        skip_group_check=True,
                    )
                # the A-half evac copies BOTH partition halves into vT
                # (rows 64:128 of cols sA are dead space) so the read
                # carries a RAW dependency on the group-closing B matmul
                # and can't be scheduled while the group is open.
                nc.vector.tensor_copy(vT[:, sA : sA + wA], pvd[:, :wA])
                ta, tb = sA // P, (sA + wA) // P
                nc.sync.dma_start_transpose(
                    vN[:, ta:tb, :], vT[0:H, sA : sA + wA]
                )
                nc.vector.tensor_copy(vS[:, ta:tb, 0:H], vN[:, ta:tb, :])
                _v_finish(sB, (H, P), pvd)

            def emit_qblock_score(part, q0r):
                """Score matmuls + exp + causal mask for one q-block.
                Returns the state the AV phase needs (wt tiles survive
                in wpool until consumed)."""
                part_q0 = 0 if part == 0 else SV
                part_len = SV if part == 0 else SP
                kc_base = 0 if part == 0 else NKC_V
                w = min(512, part_len - q0r)
                q0 = part_q0 + q0r
                if part == 0:
                    kcs = list(range(0, (q0r + w - 1) // P + 1))
                else:
                    kcs = list(range(NKC_P))

                spb = 512 // w  # score slots per PSUM bank
                cap = 2 * spb  # slots per 2-bank score group
                groups = [kcs[i : i + cap] for i in range(0, len(kcs), cap)]
                gstates = []
                for grp in groups:
                    st_ps = spsum.tile([P, 2 * 512], f32)
                    wt = wpool.tile([P, 2 * 512], bf)
                    offs = [
                        (i // spb) * 512 + (i % spb) * w for i in range(len(grp))
                    ]
                    # row-tile pairing: slot j (bank 0) together with slot
                    # j+spb (bank 1) stream CONCURRENTLY on array
                    # row-halves 0:64 / 64:128 (K=64 each).
                    for j in range(spb):
                        for half, i in enumerate(
                            i for i in (j, j + spb) if i < len(grp)
                        ):
                            kc = kc_base + grp[i]
                            lo = 64 * half
                            nc.tensor.matmul(
                                st_ps[:, offs[i] : offs[i] + w],
                                lhsT=kT[lo : lo + 64, kc * P : (kc + 1) * P],
                                rhs=qT[lo : lo + 64, q0 : q0 + w],
                                start=True,
                                stop=True,
                                tile_position=(lo, 0),
                            )
                    if 512 % w == 0:  # slots are contiguous
                        n = len(grp) * w
                        nc.scalar.activation(wt[:, 0:n], st_ps[:, 0:n], Exp)
                    else:
                        for off in offs:
                            nc.scalar.activation(
                                wt[:, off : off + w], st_ps[:, off : off + w], Exp
                            )
                    if part == 0:
                        for i, kcr in enumerate(grp):
                            d = kcr * P - q0r
                            if d >= 0:  # diagonal-band block
                                off = offs[i]
                                nc.vector.tensor_mul(
                                    wt[:, off : off + w],
                                    wt[:, off : off + w],
                                    c01_s[:, 512 - d : 512 - d + w],
                                )
                    gstates.append((grp, wt, offs))
                return (kc_base, w, q0, len(kcs), gstates)

            def emit_qblock_av(state):
                kc_base, w, q0, n_kc, gstates = state
                ot = acc.tile([H + 1, 512], f32, tag="acc", name="ot")
                ki = 0
                for grp, wt, offs in gstates:
                    for i, kcr in enumerate(grp):
                        kc = kc_base + kcr
                        nc.tensor.matmul(
                            ot[:, :w],
                            lhsT=vS[:, kc, :],
                            rhs=wt[:, offs[i] : offs[i] + w],
                            start=(ki == 0),
                            stop=(ki == n_kc - 1),
                        )
                        ki += 1
                osb = opool.tile([H + 1, 512], f32)
                nc.vector.tensor_copy(osb[:, :w], ot[:, :w])
                nc.sync.dma_start(outT_d[:, q0 : q0 + w], osb[:, :w])

            # schedule: a q-block's SCORES may be emitted once the qk
            # slices covering its queries and keys are emitted (the
            # bounced q/k mirrors arrive right behind the projections);
            # its AV needs the V tiles up to its key range.  Scores run
            # at most 2 q-blocks ahead of AVs so wt tiles stay bounded
            # and the PE interleaves score and AV streams.
            qblocks = []  # (part, q0r, need_cols)
            for part in range(2):
                part_q0 = 0 if part == 0 else SV
                part_len = SV if part == 0 else SP
                for q0r in range(0, part_len, 512):
                    w = min(512, part_len - q0r)
                    if part == 0:
                        kmax = ((q0r + w - 1) // P + 1) * P
                    else:
                        kmax = SV + SP
                    need = max(part_q0 + q0r + w, kmax)
                    qblocks.append((part, q0r, need))
            qs = qa = 0
            states = {}

            def try_emit(k_cov, v_cov):
                nonlocal qs, qa
                progress = True
                while progress:
                    progress = False
                    if (
                        qs < len(qblocks)
                        and qs < qa + 1
                        and qblocks[qs][2] <= k_cov
                    ):
                        states[qs] = emit_qblock_score(
                            qblocks[qs][0], qblocks[qs][1]
                        )
                        qs += 1
                        progress = True
                    if qa < qs and qblocks[qa][2] <= v_cov:
                        emit_qblock_av(states.pop(qa))
                        qa += 1
                        progress = True

            slice_starts = list(range(0, SVP, 512))
            v_pending = []
            k_cov = v_cov = 0
            for idx, s in enumerate(slice_starts):
                emit_qk_slice(s)
                k_cov = min(s + 512, SVP)
                try_emit(k_cov, v_cov)
                if idx < 4:
                    # PE-activity filler: bridge the DMA-bound wait for
                    # the next slice so HAM's free-running 3.4us window
                    # sees continuous busy and holds K=8/8 (a single
                    # re-throttle costs ~2x on several microseconds of
                    # matmuls; a filler costs 90-210ns and only runs
                    # when the PE would have idled anyway).
                    for _ in range(8):
                        nc.tensor.matmul(
                            warm_ps[:], lhsT=wz[:, 0:P],
                            rhs=wz[:, 0:256], start=True, stop=True,
                        )
                v_pending.append(s)
                if len(v_pending) == 2:
                    a, b = v_pending
                    if min(512, SVP - a) == min(512, SVP - b):
                        emit_v_pair(a, b)
                        v_pending = []
                    else:
                        emit_v_pair(a, None)
                        v_pending = [b]
                    v_cov = min((s + 512 if not v_pending else s), SVP)
                if idx == len(slice_starts) - 1:
                    for a in v_pending:
                        emit_v_pair(a, None)
                    v_pending = []
                    v_cov = SVP
                try_emit(k_cov, v_cov)
            assert qa == len(qblocks), (qa, qs)
    return nc


def _prepare(hidden_state, attention_masks, Wq, bq, Wk, bk, Wv, bv):
    """Host-side shard prep: sort each sequence into [valid | pad],
    pad both groups to shared multiples of 128, cast to bf16."""
    hs = np.asarray(hidden_state, dtype=np.float32)
    m = np.asarray(attention_masks)
    perms, nvs = [], []
    for b in range(B):
        mb = np.asarray(m[b]).astype(np.int64)
        perms.append(np.argsort(1 - mb, kind="stable"))
        nvs.append(int(mb.sum()))
    nps = [S - nv for nv in nvs]
    SV = max(128, -(-max(nvs) // P) * P)
    SPn = max(128, -(-max(nps) // P) * P)
    SVP = SV + SPn
    NSL = (SVP + 511) // 512
    WT = SVP - (NSL - 1) * 512

    wqk = np.ascontiguousarray(
        np.concatenate(
            [np.asarray(Wq, np.float32) / np.sqrt(H), np.asarray(Wk, np.float32)],
            axis=1,
        ).reshape(8, P, P).transpose(1, 0, 2)
    ).astype(BF)  # [p, c, m]
    wv = np.ascontiguousarray(
        np.asarray(Wv, np.float32).reshape(8, P, H).transpose(1, 0, 2)
    ).astype(BF)  # [p, c, m]
    # k takes NO bias on device: softmax is invariant to the per-query
    # constant it would add, and zero slot-pad k columns are required
    # for the kill-free masking (see module docstring).
    bqk = np.concatenate(
        [np.asarray(bq, np.float32) / np.sqrt(H), np.zeros(H, np.float32)]
    ).reshape(P, 1).astype(np.float32)

    # c01[j, 512+y] = 1.0 iff j <= y   (keep when q_rel - d >= j)
    y = np.arange(1024) - 512
    c01 = (np.arange(P)[:, None] <= y[None, :]).astype(BF)

    in_maps = []
    for b in range(B):
        nv, npd = nvs[b], nps[b]
        hs_sorted = np.zeros((NSL * 512, E), np.float32)
        hs_sorted[:nv] = hs[b][perms[b][:nv]]
        hs_sorted[SV : SV + npd] = hs[b][perms[b][nv:]]
        # full slices [128, NSF, 8, 512]:
        #   hsT[p, si, c, j] = hs_sorted[si*512+j, c*128+p]
        hsT = np.ascontiguousarray(
            hs_sorted[: (NSL - 1) * 512]
            .reshape(NSL - 1, 512, 8, P)
            .transpose(3, 0, 2, 1)
        ).astype(BF)
        # dense tail slice [128, 8, WT]
        hsTt = np.ascontiguousarray(
            hs_sorted[(NSL - 1) * 512 : (NSL - 1) * 512 + WT]
            .reshape(WT, 8, P)
            .transpose(2, 1, 0)
        ).astype(BF)
        in_maps.append(
            {
                "hsT": hsT,
                "hsTt": hsTt,
                "wqk": wqk,
                "wv": wv,
                "bqk": bqk,
                "c01": c01,
            }
        )
    return in_maps, perms, nvs, SV, SPn


def _finish_host(ot, perm, nv, SV, SPn, bv):
    """Normalize, unsort, restore bias for one batch element."""
    npd = S - nv
    den = ot[H].copy()
    # pad-part denominators carry one exp(0)=1 per slot-padded key
    den[SV:] -= SPn - npd
    with np.errstate(divide="ignore", invalid="ignore", over="ignore"):
        dev = (ot[:H] / den).T
    out = np.empty((S, H), np.float32)
    out[perm[:nv]] = dev[:nv]
    out[perm[nv:]] = dev[SV : SV + npd]
    return out + bv


def _run(inputs: dict, trace: bool = False):
    from concourse import bass_utils

    in_maps, perms, nvs, SV, SPn = _prepare(**inputs)
    key = (SV, SPn)
    if key not in _NC_CACHE:
        _NC_CACHE[key] = build_nc(SV, SPn)
    nc = _NC_CACHE[key]

    res = bass_utils.run_bass_kernel_spmd(
        nc, in_maps, core_ids=list(range(8)), trace=trace
    )

    bv = np.asarray(inputs["bv"], np.float32)
    out = np.empty((B, S, H), np.float32)
    for b in range(B):
        out[b] = _finish_host(res.results[b]["outT"], perms[b], nvs[b], SV, SPn, bv)
    return out, res


def kernel(**inputs) -> np.ndarray:
    out, _ = _run(inputs, trace=False)
    return out


# revision 29
# speedup vs baseline: 1.1316x; 1.1316x over previous
"""Trainium2 Bass kernel for nn_AttentionHead_51178830299302.

Single attention head: B=8, S=2048, E=1024, H=64, fp32 I/O, decoder
(causal) masking plus a pad-pad coupling term (padded queries attend
bidirectionally to padded keys).

Strategy:
  * Data-parallel over batch: one batch element per NeuronCore (8 cores).
  * Host-side, each sequence is stably partitioned into [valid | pad]
    positions (order preserved).  The masked softmax then decomposes
    exactly into two independent attention problems:
      - valid x valid with plain causal masking,
      - pad x pad with full bidirectional softmax (no mask),
    which skips ~60% of the S x S exp/matmul work vs. the dense mask.
  * bf16 matmul pipeline (fp32 PSUM accumulation), exp on ScalarE
    straight from PSUM.
  * HBM bandwidth (~360 GB/s, shared) binds the head of the kernel and
    a single DMA ring tops out at ~260 GB/s, so the bulk hidden-state
    stream is split across the scalar + gpsimd rings (half-slices
    each), while the SYNC ring stays empty for the latency-critical
    partition-bounce / V-transpose / output hops (rings drain FIFO, so
    a small hop queued behind bulk would stall until the bulk drains).
  * k projects with NO bias on device: softmax is invariant to the
    per-query constant <bk, q> it would add, and slot-padded keys
    (zeroed hidden state) then score exactly 0 -> exp 1, contributing
    nothing to the numerator (their v rows are 0) and a per-batch
    constant to the pad-part denominators that the host divide
    subtracts.  No kill row -> score contraction depth K=64, so score
    matmuls for two key chunks run CONCURRENTLY as PE row-tiles
    (rows 0:64 vs 64:128) on mirrored q/k copies.
  * q/k land in both partition halves: direct evacuation into one half,
    one 64-row SBUF->SBUF bounce into the other.
  * V is moved to natural [seq, head] layout with one XBAR transpose
    DMA per 512-slice; attention output is produced transposed
    [H+1, seq] with the softmax row-sums riding as an appended ones
    column of V; final divide+transpose happens on host.
  * Causal masking inside diagonal 128-blocks is one bf16 multiply
    with a constant 0/1 tril tile.

kernel(**inputs) takes the FULL unsharded fp32 inputs and returns the
FULL [8, 2048, 64] fp32 output.
"""

import numpy as np
import ml_dtypes

B, S, E, H = 8, 2048, 1024, 64
P = 128
BF = ml_dtypes.bfloat16

_NC_CACHE: dict = {}


def _patch_tile_drain():
    """The stock TileContext exit hangs every global-clock wait on a single
    Drain instruction; this container's walrus caps sync waits at 1 per
    instruction.  Split the waits across single-wait nops, and drop the
    second (post-semclear) all-engine barrier — engines halt right after,
    and NEFF re-execution only starts once every engine has halted."""
    import concourse.tile as tile
    import concourse.mybir as mybir
    from bass_rust import ScopedClock

    if getattr(tile.TileContext, "_drain_waits_split", False):
        return

    def _drain_and_barrier(self, tick_clock, wait_clock):
        nc = self.nc
        carrier = nc.sync.nop(nofuse=True)
        wait_clock.add_sem_waits(
            carrier.ins, ScopedClock({None: tick_clock.global_clock})
        )
        si = carrier.ins.sync_info
        waits = list(si.on_wait) if si and si.on_wait else []
        if len(waits) > 1:
            si.on_wait = waits[:1]
            for w in waits[1:]:
                n = nc.sync.nop(nofuse=True)
                nsi = n.ins.sync_info
                if nsi is None:
                    n.ins.sync_info = mybir.SyncInfo(on_wait=[w], on_update=[])
                else:
                    nsi.on_wait = [w]
        nc.sync.drain()
        nc.all_engine_barrier(sem_only=True)
        popped = nc._tile_sem_poison_stack.pop()
        assert popped is self._sem_poison
        nc.clear_and_free_semaphores(list(self.sems.allocated().values()))

    tile.TileContext._drain_and_barrier = _drain_and_barrier
    tile.TileContext._drain_waits_split = True


def _patch_sync_wait_split():
    """This container's walrus codegen rejects instructions carrying more
    than one sync wait.  Post-process the serialized BIR: hoist excess
    waits onto injected NoOps on the same engine, just before the
    instruction (the sequencer executes them in order, so semantics are
    preserved)."""
    import json
    import concourse.bass as bass

    if getattr(bass.Bass, "_sync_wait_split", False):
        return
    orig = bass.Bass.to_json_bytes

    def to_json_bytes(self) -> bytes:
        j = json.loads(orig(self))
        ctr = [0]

        def fix_block(blk):
            insts = blk.get("instructions")
            if not isinstance(insts, list):
                return
            out = []
            for inst in insts:
                si = inst.get("sync_info")
                ow = (si or {}).get("on_wait") or []
                if len(ow) > 1:
                    si["on_wait"] = ow[-1:]
                    for w in ow[:-1]:
                        ctr[0] += 1
                        out.append(
                            {
                                "debug": inst.get("debug", 0),
                                "engine": inst["engine"],
                                "ins": [],
                                "name": f"I-wsplit-{ctr[0]}",
                                "opcode": "NoOp",
                                "outs": [],
                                "sync_info": {"on_wait": [w], "on_update": []},
                            }
                        )
                out.append(inst)
            blk["instructions"] = out

        def rec(o):
            if isinstance(o, dict):
                if "instructions" in o:
                    fix_block(o)
                for v in o.values():
                    rec(v)
            elif isinstance(o, list):
                for v in o:
                    rec(v)

        rec(j)
        return json.dumps(j).encode()

    bass.Bass.to_json_bytes = to_json_bytes
    bass.Bass._sync_wait_split = True


def build_nc(SV: int, SP: int):
    """Build the SPMD per-core Bass program.

    Per-core DRAM tensors:
      hsT  [P, NSF, 8, 512]  bf16  full 512-col slices of the sorted,
                                   transposed hidden state
      hsTt [P, 8, WT]        bf16  the final partial slice (WT cols)
      wqk  [P, 8, P]   bf16  [Wq/sqrt(H) | Wk]
      wv   [P, 8, H]   bf16
      bqk  [P, 1]      f32   [bq/sqrt(H) ; 0]  (k takes no device bias)
      c01  [P, 1024]   bf16  tril keep-mask: c01[j, 512+y] = (j <= y)
      outT [65, SVP]   f32   rows 0..63 unnormalized output^T, row 64
                             softmax denominators (host divides)
    """
    import concourse.bass as bass
    import concourse.mybir as mybir
    import concourse.tile as tile
    from contextlib import ExitStack

    _patch_tile_drain()
    _patch_sync_wait_split()
    bf, f32 = mybir.dt.bfloat16, mybir.dt.float32
    Exp = mybir.ActivationFunctionType.Exp

    SVP = SV + SP
    NKC_V, NKC_P = SV // P, SP // P
    NT = SVP // P

    nc = bass.Bass("TRN2", target_bir_lowering=False, debug=False)
    NSL = (SVP + 511) // 512  # number of 512-col projection slices
    WT = SVP - (NSL - 1) * 512  # width of the final slice
    NSF = NSL - 1  # full slices
    hsT_d = nc.dram_tensor("hsT", [P, NSF, 8, 512], bf, kind="ExternalInput").ap()
    hsTt_d = nc.dram_tensor("hsTt", [P, 8, WT], bf, kind="ExternalInput").ap()
    wqk_d = nc.dram_tensor("wqk", [P, 8, P], bf, kind="ExternalInput").ap()
    wv_d = nc.dram_tensor("wv", [P, 8, H], bf, kind="ExternalInput").ap()
    bqk_d = nc.dram_tensor("bqk", [P, 1], f32, kind="ExternalInput").ap()
    c01_d = nc.dram_tensor("c01", [P, 1024], bf, kind="ExternalInput").ap()
    outT_d = nc.dram_tensor("outT", [H + 1, SVP], f32, kind="ExternalOutput").ap()

    with tile.TileContext(nc) as tc, ExitStack() as ctx:
        singles = ctx.enter_context(tc.tile_pool(name="singles", bufs=1))

        # PE warm-up source tile: memset first so the warm-up matmul
        # stream starts as soon as the engines come up.
        wz = singles.tile([P, 512], bf)
        nc.vector.memset(wz[:], 0.0)
        ones_t = singles.tile([1, P], bf)
        nc.vector.memset(ones_t[:], 1.0)

        wqk_s = singles.tile([P, 8, P], bf)
        wv_s = singles.tile([P, 8, H], bf)
        bqk_s = singles.tile([P, 1], f32)
        c01_s = singles.tile([P, 1024], bf)

        # q/k in BOTH partition halves (row-tile score pairing):
        # rows 0:64 = "lo" copy, rows 64:128 = "hi" copy.
        # Direct evacuation: q -> lo (PSUM rows 0:64), k -> hi (rows
        # 64:128); the other half arrives via one SBUF->SBUF bounce.
        qT = singles.tile([P, SVP], bf)
        kT = singles.tile([P, SVP], bf)

        # V in natural [seq-part, head] layout with an appended ones
        # column (row-sums of the attention weights ride along in the
        # AV matmul as output row H).
        vS = singles.tile([P, NT, H + 1], bf)
        nc.vector.memset(vS[:, :, H : H + 1], 1.0)
        vT = singles.tile([P, SVP], bf)
        # XBAR transpose needs a contiguous destination on HW; stage here,
        # then strided-copy into vS (which carries the ones column).
        vN = singles.tile([P, NT, H], bf)

        # One HWDGE ring tops out at ~260 GB/s; only two rings together
        # reach the ~360 GB/s HBM ceiling.  And HWDGE rings drain FIFO,
        # so latency-critical hops must not share a ring with bulk.
        # Split: scalar + gpsimd carry the bulk halves (A = E-chunks
        # 0:4, B = 4:8), the SYNC ring stays EMPTY for the bounce /
        # transpose / output hops.  Constants interleave into the
        # scalar stream where they are first needed.
        # All pieces use 4 KB-per-partition descriptors: the SDMA engines
        # round-robin queues at packet granularity, so unequal packet
        # sizes skew the bandwidth split.
        hsT = singles.tile([P, NSF, 8, 512], bf)
        hsTt = singles.tile([P, 8, WT], bf)
        nc.scalar.dma_start(wqk_s[:], wqk_d)
        nc.scalar.dma_start(hsT[:, 0, 0:4, :], hsT_d[:, 0, 0:4, :])
        nc.scalar.dma_start(bqk_s[:], bqk_d)
        nc.scalar.dma_start(c01_s[:], c01_d)
        nc.scalar.dma_start(hsT[:, 1, 0:4, :], hsT_d[:, 1, 0:4, :])
        nc.scalar.dma_start(wv_s[:], wv_d)
        for si in range(2, NSF):
            nc.scalar.dma_start(hsT[:, si, 0:4, :], hsT_d[:, si, 0:4, :])
        for si in range(NSF):
            nc.gpsimd.dma_start(hsT[:, si, 4:8, :], hsT_d[:, si, 4:8, :])
        nc.gpsimd.dma_start(hsTt[:, 0:4, :], hsTt_d[:, 0:4, :])
        nc.gpsimd.dma_start(hsTt[:, 4:8, :], hsTt_d[:, 4:8, :])

        def hs_chunk(si, c, w):
            if si < NSF:
                return hsT[:, si, c, :w]
            return hsTt[:, c, :w]

        # ------- interleaved projections + attention -------
        # Emission order interleaves projection slices with attention
        # q-blocks whose inputs are already covered, so the PE stream has
        # no phase barrier and HAM stays warm.
        with tc.tile_pool(name="pp", bufs=1, space="PSUM") as pp, \
             tc.tile_pool(name="acc", bufs=2, space="PSUM") as acc, \
             tc.tile_pool(name="spsum", bufs=2, space="PSUM") as spsum, \
             tc.tile_pool(name="wpool", bufs=8) as wpool, \
             tc.tile_pool(name="opool", bufs=3) as opool, \
             tc.tile_pool(name="warmp", bufs=1, space="PSUM") as warmp:

            # ~12 cold N=256 matmuls span engine start -> first hsT
            # slice landing, front-loading PE activity while the DMA
            # streams; real work follows immediately in the PE FIFO.
            warm_ps = warmp.tile([P, 256], f32)
            for _ in range(12):
                nc.tensor.matmul(
                    warm_ps[:], lhsT=wz[:, 0:P], rhs=wz[:, 0:256],
                    start=True, stop=True,
                )

            # Contraction-chunk order: the B-half (gpsimd ring) lands
            # slightly before the A-half, so start with chunks 4:8.
            CORD = [4, 5, 6, 7, 0, 1, 2, 3]

            def emit_qk_slice(sb):
                si = sb // 512
                w = min(512, SVP - sb)
                ps = pp.tile([P, 512], f32)
                for ci, c in enumerate(CORD):
                    nc.tensor.matmul(
                        ps[:, :w],
                        lhsT=wqk_s[:, c, :],
                        rhs=hs_chunk(si, c, w),
                        start=(ci == 0),
                        stop=(ci == 7),
                    )
                # q -> partitions 0:64 (bias add), k -> partitions 64:128
                # (bias row is 0 there; the add is just the bf16 cast).
                nc.vector.tensor_scalar_add(
                    qT[0:64, sb : sb + w], ps[0:64, :w], bqk_s[0:64, 0:1]
                )
                nc.vector.tensor_scalar_add(
                    kT[64:128, sb : sb + w], ps[64:128, :w], bqk_s[64:128, 0:1]
                )
                # mirror each into the other partition half (engines
                # cannot shift partitions; DMA can).  sync = the empty
                # latency ring: lands ~0.4us after the evacuation.
                nc.sync.dma_start(kT[0:64, sb : sb + w], kT[64:128, sb : sb + w])
                nc.sync.dma_start(qT[64:128, sb : sb + w], qT[0:64, sb : sb + w])

            def _v_finish(s, rows, pvd):
                w = min(512, SVP - s)
                cp = nc.vector.tensor_copy(vT[rows[0] : rows[1], s : s + w],
                                           pvd[rows[0] : rows[1], :w])
                ta, tb = s // P, (s + w) // P
                nc.sync.dma_start_transpose(
                    vN[:, ta:tb, :], vT[rows[0] : rows[1], s : s + w]
                )
                nc.vector.tensor_copy(vS[:, ta:tb, 0:H], vN[:, ta:tb, :])
                return cp

            def emit_v_pair(sA, sB):
                # V projection for two 512-slices concurrently via PE
                # column tiling: slice A in array cols 0:64 -> PSUM rows
                # 0:64, slice B in cols 64:128 -> PSUM rows 64:128.  A
                # K=1 zero matmul opens the accumulation group for the
                # whole bank (per-chain start=True would clear the
                # sibling chain's has_written bits).
                pvd = acc.tile([P, 512], f32, tag="acc", name="pvd")
                wA = min(512, SVP - sA)
                if sB is None:
                    for c in range(8):
                        nc.tensor.matmul(
                            pvd[0:H, :wA],
                            lhsT=wv_s[:, c, :],
                            rhs=hs_chunk(sA // 512, c, wA),
                            start=(c == 0),
                            stop=(c == 7),
                        )
                    _v_finish(sA, (0, H), pvd)
                    return
                wB = min(512, SVP - sB)
                nc.tensor.matmul(
                    pvd[:, 0:512], lhsT=ones_t[:], rhs=wz[0:1, 0:512],
                    start=True, stop=True, skip_group_check=True,
                )
                for c in range(8):
                    nc.tensor.matmul(
                        pvd[0:H, :wA],
                        lhsT=wv_s[:, c, :],
                        rhs=hs_chunk(sA // 512, c, wA),
                        start=False, stop=(c == 7), tile_position=(0, 0),
                        skip_group_check=True,
                    )
                    nc.tensor.matmul(
                        pvd[H:P, :wB],
                        lhsT=wv_s[:, c, :],
                        rhs=hs_chunk(sB // 512, c, wB),
                        start=False, stop=(c == 7), tile_position=(0, H),
                        skip_group_check=True,
                    )
                # the A-half evac copies BOTH partition halves into vT
                # (rows 64:128 of cols sA are dead space) so the read
                # carries a RAW dependency on the group-closing B matmul
                # and can't be scheduled while the group is open.
                nc.vector.tensor_copy(vT[:, sA : sA + wA], pvd[:, :wA])
                ta, tb = sA // P, (sA + wA) // P
                nc.sync.dma_start_transpose(
                    vN[:, ta:tb, :], vT[0:H, sA : sA + wA]
                )
                nc.vector.tensor_copy(vS[:, ta:tb, 0:H], vN[:, ta:tb, :])
                _v_finish(sB, (H, P), pvd)

            def emit_qblock_score(part, q0r):
                """Score matmuls + exp + causal mask for one q-block.
                Returns the state the AV phase needs (wt tiles survive
                in wpool until consumed)."""
                part_q0 = 0 if part == 0 else SV
                part_len = SV if part == 0 else SP
                kc_base = 0 if part == 0 else NKC_V
                w = min(512, part_len - q0r)
                q0 = part_q0 + q0r
                if part == 0:
                    kcs = list(range(0, (q0r + w - 1) // P + 1))
                else:
                    kcs = list(range(NKC_P))

                spb = 512 // w  # score slots per PSUM bank
                cap = 2 * spb  # slots per 2-bank score group
                groups = [kcs[i : i + cap] for i in range(0, len(kcs), cap)]
                gstates = []
                for grp in groups:
                    st_ps = spsum.tile([P, 2 * 512], f32)
                    wt = wpool.tile([P, 2 * 512], bf)
                    offs = [
                        (i // spb) * 512 + (i % spb) * w for i in range(len(grp))
                    ]
                    # row-tile pairing: slot j (bank 0) together with slot
                    # j+spb (bank 1) stream CONCURRENTLY on array
                    # row-halves 0:64 / 64:128 (K=64 each).
                    for j in range(spb):
                        for half, i in enumerate(
                            i for i in (j, j + spb) if i < len(grp)
                        ):
                            kc = kc_base + grp[i]
                            lo = 64 * half
                            nc.tensor.matmul(
                                st_ps[:, offs[i] : offs[i] + w],
                                lhsT=kT[lo : lo + 64, kc * P : (kc + 1) * P],
                                rhs=qT[lo : lo + 64, q0 : q0 + w],
                                start=True,
                                stop=True,
                                tile_position=(lo, 0),
                            )
                    if 512 % w == 0:  # slots are contiguous
                        n = len(grp) * w
                        nc.scalar.activation(wt[:, 0:n], st_ps[:, 0:n], Exp)
                    else:
                        for off in offs:
                            nc.scalar.activation(
                                wt[:, off : off + w], st_ps[:, off : off + w], Exp
                            )
                    if part == 0:
                        for i, kcr in enumerate(grp):
                            d = kcr * P - q0r
                            if d >= 0:  # diagonal-band block
                                off = offs[i]
                                nc.vector.tensor_mul(
                                    wt[:, off : off + w],
                                    wt[:, off : off + w],
                                    c01_s[:, 512 - d : 512 - d + w],
                                )
                    gstates.append((grp, wt, offs))
                return (kc_base, w, q0, len(kcs), gstates)

            def emit_qblock_av(state):
                kc_base, w, q0, n_kc, gstates = state
                ot = acc.tile([H + 1, 512], f32, tag="acc", name="ot")
                ki = 0
                for grp, wt, offs in gstates:
                    for i, kcr in enumerate(grp):
                        kc = kc_base + kcr
                        nc.tensor.matmul(
                            ot[:, :w],
                            lhsT=vS[:, kc, :],
                            rhs=wt[:, offs[i] : offs[i] + w],
                            start=(ki == 0),
                            stop=(ki == n_kc - 1),
                        )
                        ki += 1
                osb = opool.tile([H + 1, 512], f32)
                nc.vector.tensor_copy(osb[:, :w], ot[:, :w])
                nc.sync.dma_start(outT_d[:, q0 : q0 + w], osb[:, :w])

            # schedule: a q-block's SCORES may be emitted once the qk
            # slices covering its queries and keys are emitted (the
            # bounced q/k mirrors arrive right behind the projections);
            # its AV needs the V tiles up to its key range.  Scores run
            # at most 2 q-blocks ahead of AVs so wt tiles stay bounded
            # and the PE interleaves score and AV streams.
            qblocks = []  # (part, q0r, need_cols)
            for part in range(2):
                part_q0 = 0 if part == 0 else SV
                part_len = SV if part == 0 else SP
                for q0r in range(0, part_len, 512):
                    w = min(512, part_len - q0r)
                    if part == 0:
                        kmax = ((q0r + w - 1) // P + 1) * P
                    else:
                        kmax = SV + SP
                    need = max(part_q0 + q0r + w, kmax)
                    qblocks.append((part, q0r, need))
            qs = qa = 0
            states = {}

            def try_emit(k_cov, v_cov):
                nonlocal qs, qa
                progress = True
                while progress:
                    progress = False
                    if (
                        qs < len(qblocks)
                        and qs < qa + 2
                        and qblocks[qs][2] <= k_cov
                    ):
                        states[qs] = emit_qblock_score(
                            qblocks[qs][0], qblocks[qs][1]
                        )
                        qs += 1
                        progress = True
                    if qa < qs and qblocks[qa][2] <= v_cov:
                        emit_qblock_av(states.pop(qa))
                        qa += 1
                        progress = True

            slice_starts = list(range(0, SVP, 512))
            v_pending = []
            k_cov = v_cov = 0
            for idx, s in enumerate(slice_starts):
                emit_qk_slice(s)
                k_cov = min(s + 512, SVP)
                try_emit(k_cov, v_cov)
                v_pending.append(s)
                if len(v_pending) == 2:
                    a, b = v_pending
                    if min(512, SVP - a) == min(512, SVP - b):
                        emit_v_pair(a, b)
                        v_pending = []
                    else:
                        emit_v_pair(a, None)
                        v_pending = [b]
                    v_cov = min((s + 512 if not v_pending else s), SVP)
                if idx == len(slice_starts) - 1:
                    for a in v_pending:
                        emit_v_pair(a, None)
                    v_pending = []
                    v_cov = SVP
                try_emit(k_cov, v_cov)
            assert qa == len(qblocks), (qa, qs)
    return nc


def _prepare(hidden_state, attention_masks, Wq, bq, Wk, bk, Wv, bv):
    """Host-side shard prep: sort each sequence into [valid | pad],
    pad both groups to shared multiples of 128, cast to bf16."""
    hs = np.asarray(hidden_state, dtype=np.float32)
    m = np.asarray(attention_masks)
    perms, nvs = [], []
    for b in range(B):
        mb = np.asarray(m[b]).astype(np.int64)
        perms.append(np.argsort(1 - mb, kind="stable"))
        nvs.append(int(mb.sum()))
    nps = [S - nv for nv in nvs]
    SV = max(128, -(-max(nvs) // P) * P)
    SPn = max(128, -(-max(nps) // P) * P)
    SVP = SV + SPn
    NSL = (SVP + 511) // 512
    WT = SVP - (NSL - 1) * 512

    wqk = np.ascontiguousarray(
        np.concatenate(
            [np.asarray(Wq, np.float32) / np.sqrt(H), np.asarray(Wk, np.float32)],
            axis=1,
        ).reshape(8, P, P).transpose(1, 0, 2)
    ).astype(BF)  # [p, c, m]
    wv = np.ascontiguousarray(
        np.asarray(Wv, np.float32).reshape(8, P, H).transpose(1, 0, 2)
    ).astype(BF)  # [p, c, m]
    # k takes NO bias on device: softmax is invariant to the per-query
    # constant it would add, and zero slot-pad k columns are required
    # for the kill-free masking (see module docstring).
    bqk = np.concatenate(
        [np.asarray(bq, np.float32) / np.sqrt(H), np.zeros(H, np.float32)]
    ).reshape(P, 1).astype(np.float32)

    # c01[j, 512+y] = 1.0 iff j <= y   (keep when q_rel - d >= j)
    y = np.arange(1024) - 512
    c01 = (np.arange(P)[:, None] <= y[None, :]).astype(BF)

    in_maps = []
    for b in range(B):
        nv, npd = nvs[b], nps[b]
        hs_sorted = np.zeros((NSL * 512, E), np.float32)
        hs_sorted[:nv] = hs[b][perms[b][:nv]]
        hs_sorted[SV : SV + npd] = hs[b][perms[b][nv:]]
        # full slices [128, NSF, 8, 512]:
        #   hsT[p, si, c, j] = hs_sorted[si*512+j, c*128+p]
        hsT = np.ascontiguousarray(
            hs_sorted[: (NSL - 1) * 512]
            .reshape(NSL - 1, 512, 8, P)
            .transpose(3, 0, 2, 1)
        ).astype(BF)
        # dense tail slice [128, 8, WT]
        hsTt = np.ascontiguousarray(
            hs_sorted[(NSL - 1) * 512 : (NSL - 1) * 512 + WT]
            .reshape(WT, 8, P)
            .transpose(2, 1, 0)
        ).astype(BF)
        in_maps.append(
            {
                "hsT": hsT,
                "hsTt": hsTt,
                "wqk": wqk,
                "wv": wv,
                "bqk": bqk,
                "c01": c01,
            }
        )
    return in_maps, perms, nvs, SV, SPn


def _finish_host(ot, perm, nv, SV, SPn, bv):
    """Normalize, unsort, restore bias for one batch element."""
    npd = S - nv
    den = ot[H].copy()
    # pad-part denominators carry one exp(0)=1 per slot-padded key
    den[SV:] -= SPn - npd
    with np.errstate(divide="ignore", invalid="ignore", over="ignore"):
        dev = (ot[:H] / den).T
    out = np.empty((S, H), np.float32)
    out[perm[:nv]] = dev[:nv]
    out[perm[nv:]] = dev[SV : SV + npd]
    return out + bv


def _run(inputs: dict, trace: bool = False):
    from concourse import bass_utils

    in_maps, perms, nvs, SV, SPn = _prepare(**inputs)
    key = (SV, SPn)
    if key not in _NC_CACHE:
        _NC_CACHE[key] = build_nc(SV, SPn)
    nc = _NC_CACHE[key]

    res = bass_utils.run_bass_kernel_spmd(
        nc, in_maps, core_ids=list(range(8)), trace=trace
    )

    bv = np.asarray(inputs["bv"], np.float32)
    out = np.empty((B, S, H), np.float32)
    for b in range(B):
        out[b] = _finish_host(res.results[b]["outT"], perms[b], nvs[b], SV, SPn, bv)
    return out, res


def kernel(**inputs) -> np.ndarray:
    out, _ = _run(inputs, trace=False)
    return out


# revision 31
# speedup vs baseline: 1.1331x; 1.0013x over previous
"""Trainium2 Bass kernel for nn_AttentionHead_51178830299302.

Single attention head: B=8, S=2048, E=1024, H=64, fp32 I/O, decoder
(causal) masking plus a pad-pad coupling term (padded queries attend
bidirectionally to padded keys).

Strategy:
  * Data-parallel over batch: one batch element per NeuronCore (8 cores).
  * Host-side, each sequence is stably partitioned into [valid | pad]
    positions (order preserved).  The masked softmax then decomposes
    exactly into two independent attention problems:
      - valid x valid with plain causal masking,
      - pad x pad with full bidirectional softmax (no mask),
    which skips ~60% of the S x S exp/matmul work vs. the dense mask.
  * bf16 matmul pipeline (fp32 PSUM accumulation), exp on ScalarE
    straight from PSUM.
  * HBM bandwidth (~360 GB/s, shared) binds the head of the kernel and
    a single DMA ring tops out at ~260 GB/s, so the bulk hidden-state
    stream is split across the scalar + gpsimd rings (half-slices
    each), while the SYNC ring stays empty for the latency-critical
    partition-bounce / V-transpose / output hops (rings drain FIFO, so
    a small hop queued behind bulk would stall until the bulk drains).
  * k projects with NO bias on device: softmax is invariant to the
    per-query constant <bk, q> it would add, and slot-padded keys
    (zeroed hidden state) then score exactly 0 -> exp 1, contributing
    nothing to the numerator (their v rows are 0) and a per-batch
    constant to the pad-part denominators that the host divide
    subtracts.  No kill row -> score contraction depth K=64, so score
    matmuls for two key chunks run CONCURRENTLY as PE row-tiles
    (rows 0:64 vs 64:128) on mirrored q/k copies.
  * q/k land in both partition halves: direct evacuation into one half,
    one 64-row SBUF->SBUF bounce into the other.
  * V is moved to natural [seq, head] layout with one XBAR transpose
    DMA per 512-slice; attention output is produced transposed
    [H+1, seq] with the softmax row-sums riding as an appended ones
    column of V; final divide+transpose happens on host.
  * Causal masking inside diagonal 128-blocks is one bf16 multiply
    with a constant 0/1 tril tile.

kernel(**inputs) takes the FULL unsharded fp32 inputs and returns the
FULL [8, 2048, 64] fp32 output.
"""

import numpy as np
import ml_dtypes

B, S, E, H = 8, 2048, 1024, 64
P = 128
BF = ml_dtypes.bfloat16

_NC_CACHE: dict = {}


def _patch_tile_drain():
    """The stock TileContext exit hangs every global-clock wait on a single
    Drain instruction; this container's walrus caps sync waits at 1 per
    instruction.  Split the waits across single-wait nops, and drop the
    second (post-semclear) all-engine barrier — engines halt right after,
    and NEFF re-execution only starts once every engine has halted."""
    import concourse.tile as tile
    import concourse.mybir as mybir
    from bass_rust import ScopedClock

    if getattr(tile.TileContext, "_drain_waits_split", False):
        return

    def _drain_and_barrier(self, tick_clock, wait_clock):
        nc = self.nc
        carrier = nc.sync.nop(nofuse=True)
        wait_clock.add_sem_waits(
            carrier.ins, ScopedClock({None: tick_clock.global_clock})
        )
        si = carrier.ins.sync_info
        waits = list(si.on_wait) if si and si.on_wait else []
        if len(waits) > 1:
            si.on_wait = waits[:1]
            for w in waits[1:]:
                n = nc.sync.nop(nofuse=True)
                nsi = n.ins.sync_info
                if nsi is None:
                    n.ins.sync_info = mybir.SyncInfo(on_wait=[w], on_update=[])
                else:
                    nsi.on_wait = [w]
        nc.sync.drain()
        nc.all_engine_barrier(sem_only=True)
        popped = nc._tile_sem_poison_stack.pop()
        assert popped is self._sem_poison
        nc.clear_and_free_semaphores(list(self.sems.allocated().values()))

    tile.TileContext._drain_and_barrier = _drain_and_barrier
    tile.TileContext._drain_waits_split = True


def _patch_sync_wait_split():
    """This container's walrus codegen rejects instructions carrying more
    than one sync wait.  Post-process the serialized BIR: hoist excess
    waits onto injected NoOps on the same engine, just before the
    instruction (the sequencer executes them in order, so semantics are
    preserved)."""
    import json
    import concourse.bass as bass

    if getattr(bass.Bass, "_sync_wait_split", False):
        return
    orig = bass.Bass.to_json_bytes

    def to_json_bytes(self) -> bytes:
        j = json.loads(orig(self))
        ctr = [0]

        def fix_block(blk):
            insts = blk.get("instructions")
            if not isinstance(insts, list):
                return
            out = []
            for inst in insts:
                si = inst.get("sync_info")
                ow = (si or {}).get("on_wait") or []
                if len(ow) > 1:
                    si["on_wait"] = ow[-1:]
                    for w in ow[:-1]:
                        ctr[0] += 1
                        out.append(
                            {
                                "debug": inst.get("debug", 0),
                                "engine": inst["engine"],
                                "ins": [],
                                "name": f"I-wsplit-{ctr[0]}",
                                "opcode": "NoOp",
                                "outs": [],
                                "sync_info": {"on_wait": [w], "on_update": []},
                            }
                        )
                out.append(inst)
            blk["instructions"] = out

        def rec(o):
            if isinstance(o, dict):
                if "instructions" in o:
                    fix_block(o)
                for v in o.values():
                    rec(v)
            elif isinstance(o, list):
                for v in o:
                    rec(v)

        rec(j)
        return json.dumps(j).encode()

    bass.Bass.to_json_bytes = to_json_bytes
    bass.Bass._sync_wait_split = True


def build_nc(SV: int, SP: int):
    """Build the SPMD per-core Bass program.

    Per-core DRAM tensors:
      hsT  [P, NSF, 8, 512]  bf16  full 512-col slices of the sorted,
                                   transposed hidden state
      hsTt [P, 8, WT]        bf16  the final partial slice (WT cols)
      wqk  [P, 8, P]   bf16  [Wq/sqrt(H) | Wk]
      wv   [P, 8, H]   bf16
      bqk  [P, 1]      f32   [bq/sqrt(H) ; 0]  (k takes no device bias)
      c01  [P, 1024]   bf16  tril keep-mask: c01[j, 512+y] = (j <= y)
      outT [65, SVP]   f32   rows 0..63 unnormalized output^T, row 64
                             softmax denominators (host divides)
    """
    import concourse.bass as bass
    import concourse.mybir as mybir
    import concourse.tile as tile
    from contextlib import ExitStack

    _patch_tile_drain()
    _patch_sync_wait_split()
    bf, f32 = mybir.dt.bfloat16, mybir.dt.float32
    Exp = mybir.ActivationFunctionType.Exp

    SVP = SV + SP
    NKC_V, NKC_P = SV // P, SP // P
    NT = SVP // P

    nc = bass.Bass("TRN2", target_bir_lowering=False, debug=False)
    NSL = (SVP + 511) // 512  # number of 512-col projection slices
    WT = SVP - (NSL - 1) * 512  # width of the final slice
    NSF = NSL - 1  # full slices
    hsT_d = nc.dram_tensor("hsT", [P, NSF, 8, 512], bf, kind="ExternalInput").ap()
    hsTt_d = nc.dram_tensor("hsTt", [P, 8, WT], bf, kind="ExternalInput").ap()
    wqk_d = nc.dram_tensor("wqk", [P, 8, P], bf, kind="ExternalInput").ap()
    wv_d = nc.dram_tensor("wv", [P, 8, H], bf, kind="ExternalInput").ap()
    bqk_d = nc.dram_tensor("bqk", [P, 1], f32, kind="ExternalInput").ap()
    c01_d = nc.dram_tensor("c01", [P, 1024], bf, kind="ExternalInput").ap()
    outT_d = nc.dram_tensor("outT", [H + 1, SVP], f32, kind="ExternalOutput").ap()

    with tile.TileContext(nc) as tc, ExitStack() as ctx:
        singles = ctx.enter_context(tc.tile_pool(name="singles", bufs=1))

        # PE warm-up source tile: memset first so the warm-up matmul
        # stream starts as soon as the engines come up.
        wz = singles.tile([P, 512], bf)
        nc.vector.memset(wz[:], 0.0)
        ones_t = singles.tile([1, P], bf)
        nc.vector.memset(ones_t[:], 1.0)

        wqk_s = singles.tile([P, 8, P], bf)
        wv_s = singles.tile([P, 8, H], bf)
        bqk_s = singles.tile([P, 1], f32)
        c01_s = singles.tile([P, 1024], bf)

        # q/k in BOTH partition halves (row-tile score pairing):
        # rows 0:64 = "lo" copy, rows 64:128 = "hi" copy.
        # Direct evacuation: q -> lo (PSUM rows 0:64), k -> hi (rows
        # 64:128); the other half arrives via one SBUF->SBUF bounce.
        qT = singles.tile([P, SVP], bf)
        kT = singles.tile([P, SVP], bf)

        # V in natural [seq-part, head] layout with an appended ones
        # column (row-sums of the attention weights ride along in the
        # AV matmul as output row H).
        vS = singles.tile([P, NT, H + 1], bf)
        nc.vector.memset(vS[:, :, H : H + 1], 1.0)
        vT = singles.tile([P, SVP], bf)
        # XBAR transpose needs a contiguous destination on HW; stage here,
        # then strided-copy into vS (which carries the ones column).
        vN = singles.tile([P, NT, H], bf)

        # One HWDGE ring tops out at ~260 GB/s; only two rings together
        # reach the ~360 GB/s HBM ceiling.  And HWDGE rings drain FIFO,
        # so latency-critical hops must not share a ring with bulk.
        # Split: scalar + gpsimd carry the bulk halves (A = E-chunks
        # 0:4, B = 4:8), the SYNC ring stays EMPTY for the bounce /
        # transpose / output hops.  Constants interleave into the
        # scalar stream where they are first needed.
        # All pieces use 4 KB-per-partition descriptors: the SDMA engines
        # round-robin queues at packet granularity, so unequal packet
        # sizes skew the bandwidth split.
        hsT = singles.tile([P, NSF, 8, 512], bf)
        hsTt = singles.tile([P, 8, WT], bf)
        nc.scalar.dma_start(wqk_s[:], wqk_d)
        nc.scalar.dma_start(hsT[:, 0, 0:4, :], hsT_d[:, 0, 0:4, :])
        nc.scalar.dma_start(bqk_s[:], bqk_d)
        nc.scalar.dma_start(c01_s[:], c01_d)
        nc.scalar.dma_start(hsT[:, 1, 0:4, :], hsT_d[:, 1, 0:4, :])
        nc.scalar.dma_start(wv_s[:], wv_d)
        for si in range(2, NSF):
            nc.scalar.dma_start(hsT[:, si, 0:4, :], hsT_d[:, si, 0:4, :])
        for si in range(NSF):
            nc.gpsimd.dma_start(hsT[:, si, 4:8, :], hsT_d[:, si, 4:8, :])
        nc.gpsimd.dma_start(hsTt[:, 0:4, :], hsTt_d[:, 0:4, :])
        nc.gpsimd.dma_start(hsTt[:, 4:8, :], hsTt_d[:, 4:8, :])

        def hs_chunk(si, c, w):
            if si < NSF:
                return hsT[:, si, c, :w]
            return hsTt[:, c, :w]

        # ------- interleaved projections + attention -------
        # Emission order interleaves projection slices with attention
        # q-blocks whose inputs are already covered, so the PE stream has
        # no phase barrier and HAM stays warm.
        with tc.tile_pool(name="pp", bufs=1, space="PSUM") as pp, \
             tc.tile_pool(name="acc", bufs=2, space="PSUM") as acc, \
             tc.tile_pool(name="spsum", bufs=2, space="PSUM") as spsum, \
             tc.tile_pool(name="wpool", bufs=12) as wpool, \
             tc.tile_pool(name="opool", bufs=3) as opool, \
             tc.tile_pool(name="warmp", bufs=1, space="PSUM") as warmp:

            # ~12 cold N=256 matmuls span engine start -> first hsT
            # slice landing, front-loading PE activity while the DMA
            # streams; real work follows immediately in the PE FIFO.
            warm_ps = warmp.tile([P, 256], f32)
            for _ in range(12):
                nc.tensor.matmul(
                    warm_ps[:], lhsT=wz[:, 0:P], rhs=wz[:, 0:256],
                    start=True, stop=True,
                )

            # Contraction-chunk order: the B-half (gpsimd ring) lands
            # slightly before the A-half, so start with chunks 4:8.
            CORD = [4, 5, 6, 7, 0, 1, 2, 3]

            def emit_qk_slice(sb):
                si = sb // 512
                w = min(512, SVP - sb)
                ps = pp.tile([P, 512], f32)
                for ci, c in enumerate(CORD):
                    nc.tensor.matmul(
                        ps[:, :w],
                        lhsT=wqk_s[:, c, :],
                        rhs=hs_chunk(si, c, w),
                        start=(ci == 0),
                        stop=(ci == 7),
                    )
                # q -> partitions 0:64 (bias add), k -> partitions 64:128
                # (bias row is 0 there; the add is just the bf16 cast).
                nc.vector.tensor_scalar_add(
                    qT[0:64, sb : sb + w], ps[0:64, :w], bqk_s[0:64, 0:1]
                )
                nc.vector.tensor_scalar_add(
                    kT[64:128, sb : sb + w], ps[64:128, :w], bqk_s[64:128, 0:1]
                )
                # mirror each into the other partition half (engines
                # cannot shift partitions; DMA can).  sync = the empty
                # latency ring: lands ~0.4us after the evacuation.
                nc.sync.dma_start(kT[0:64, sb : sb + w], kT[64:128, sb : sb + w])
                nc.sync.dma_start(qT[64:128, sb : sb + w], qT[0:64, sb : sb + w])

            def _v_finish(s, rows, pvd):
                w = min(512, SVP - s)
                cp = nc.vector.tensor_copy(vT[rows[0] : rows[1], s : s + w],
                                           pvd[rows[0] : rows[1], :w])
                ta, tb = s // P, (s + w) // P
                nc.sync.dma_start_transpose(
                    vN[:, ta:tb, :], vT[rows[0] : rows[1], s : s + w]
                )
                nc.vector.tensor_copy(vS[:, ta:tb, 0:H], vN[:, ta:tb, :])
                return cp

            def emit_v_pair(sA, sB):
                # V projection for two 512-slices concurrently via PE
                # column tiling: slice A in array cols 0:64 -> PSUM rows
                # 0:64, slice B in cols 64:128 -> PSUM rows 64:128.  A
                # K=1 zero matmul opens the accumulation group for the
                # whole bank (per-chain start=True would clear the
                # sibling chain's has_written bits).
                pvd = acc.tile([P, 512], f32, tag="acc", name="pvd")
                wA = min(512, SVP - sA)
                if sB is None:
                    for c in range(8):
                        nc.tensor.matmul(
                            pvd[0:H, :wA],
                            lhsT=wv_s[:, c, :],
                            rhs=hs_chunk(sA // 512, c, wA),
                            start=(c == 0),
                            stop=(c == 7),
                        )
                    _v_finish(sA, (0, H), pvd)
                    return
                wB = min(512, SVP - sB)
                nc.tensor.matmul(
                    pvd[:, 0:512], lhsT=ones_t[:], rhs=wz[0:1, 0:512],
                    start=True, stop=True, skip_group_check=True,
                )
                for c in range(8):
                    nc.tensor.matmul(
                        pvd[0:H, :wA],
                        lhsT=wv_s[:, c, :],
                        rhs=hs_chunk(sA // 512, c, wA),
                        start=False, stop=(c == 7), tile_position=(0, 0),
                        skip_group_check=True,
                    )
                    nc.tensor.matmul(
                        pvd[H:P, :wB],
                        lhsT=wv_s[:, c, :],
                        rhs=hs_chunk(sB // 512, c, wB),
                        start=False, stop=(c == 7), tile_position=(0, H),
                        skip_group_check=True,
                    )
                # the A-half evac copies BOTH partition halves into vT
                # (rows 64:128 of cols sA are dead space) so the read
                # carries a RAW dependency on the group-closing B matmul
                # and can't be scheduled while the group is open.
                nc.vector.tensor_copy(vT[:, sA : sA + wA], pvd[:, :wA])
                ta, tb = sA // P, (sA + wA) // P
                nc.sync.dma_start_transpose(
                    vN[:, ta:tb, :], vT[0:H, sA : sA + wA]
                )
                nc.vector.tensor_copy(vS[:, ta:tb, 0:H], vN[:, ta:tb, :])
                _v_finish(sB, (H, P), pvd)

            def emit_qblock_score(part, q0r):
                """Score matmuls + exp + causal mask for one q-block.
                Returns the state the AV phase needs (wt tiles survive
                in wpool until consumed)."""
                part_q0 = 0 if part == 0 else SP
                part_len = SP if part == 0 else SV
                kc_base = 0 if part == 0 else NKC_P
                w = min(512, part_len - q0r)
                q0 = part_q0 + q0r
                if part == 0:
                    kcs = list(range(NKC_P))
                else:
                    kcs = list(range(0, (q0r + w - 1) // P + 1))

                spb = 512 // w  # score slots per PSUM bank
                cap = 2 * spb  # slots per 2-bank score group
                groups = [kcs[i : i + cap] for i in range(0, len(kcs), cap)]
                gstates = []
                for grp in groups:
                    st_ps = spsum.tile([P, 2 * 512], f32)
                    wt = wpool.tile([P, 2 * 512], bf)
                    offs = [
                        (i // spb) * 512 + (i % spb) * w for i in range(len(grp))
                    ]
                    # row-tile pairing: slot j (bank 0) together with slot
                    # j+spb (bank 1) stream CONCURRENTLY on array
                    # row-halves 0:64 / 64:128 (K=64 each).
                    for j in range(spb):
                        for half, i in enumerate(
                            i for i in (j, j + spb) if i < len(grp)
                        ):
                            kc = kc_base + grp[i]
                            lo = 64 * half
                            nc.tensor.matmul(
                                st_ps[:, offs[i] : offs[i] + w],
                                lhsT=kT[lo : lo + 64, kc * P : (kc + 1) * P],
                                rhs=qT[lo : lo + 64, q0 : q0 + w],
                                start=True,
                                stop=True,
                                tile_position=(lo, 0),
                            )
                    if 512 % w == 0:  # slots are contiguous
                        n = len(grp) * w
                        nc.scalar.activation(wt[:, 0:n], st_ps[:, 0:n], Exp)
                    else:
                        for off in offs:
                            nc.scalar.activation(
                                wt[:, off : off + w], st_ps[:, off : off + w], Exp
                            )
                    if part == 1:
                        for i, kcr in enumerate(grp):
                            d = kcr * P - q0r
                            if d >= 0:  # diagonal-band block
                                off = offs[i]
                                nc.vector.tensor_mul(
                                    wt[:, off : off + w],
                                    wt[:, off : off + w],
                                    c01_s[:, 512 - d : 512 - d + w],
                                )
                    gstates.append((grp, wt, offs))
                return (kc_base, w, q0, len(kcs), gstates)

            def emit_qblock_av(state):
                kc_base, w, q0, n_kc, gstates = state
                ot = acc.tile([H + 1, 512], f32, tag="acc", name="ot")
                ki = 0
                for grp, wt, offs in gstates:
                    for i, kcr in enumerate(grp):
                        kc = kc_base + kcr
                        nc.tensor.matmul(
                            ot[:, :w],
                            lhsT=vS[:, kc, :],
                            rhs=wt[:, offs[i] : offs[i] + w],
                            start=(ki == 0),
                            stop=(ki == n_kc - 1),
                        )
                        ki += 1
                osb = opool.tile([H + 1, 512], f32)
                nc.vector.tensor_copy(osb[:, :w], ot[:, :w])
                nc.sync.dma_start(outT_d[:, q0 : q0 + w], osb[:, :w])

            # schedule: a q-block's SCORES may be emitted once the qk
            # slices covering its queries and keys are emitted (the
            # bounced q/k mirrors arrive right behind the projections);
            # its AV needs the V tiles up to its key range.  Scores run
            # at most 2 q-blocks ahead of AVs so wt tiles stay bounded
            # and the PE interleaves score and AV streams.
            qblocks = []  # (part, q0r, need_cols)
            for part in range(2):
                part_len = SP if part == 0 else SV
                for q0r in range(0, part_len, 512):
                    w = min(512, part_len - q0r)
                    if part == 0:
                        need = SP
                    else:
                        need = SP + ((q0r + w - 1) // P + 1) * P
                    qblocks.append((part, q0r, need))
            qs = qa = 0
            states = {}

            def try_emit(k_cov, v_cov):
                nonlocal qs, qa
                progress = True
                while progress:
                    progress = False
                    if (
                        qs < len(qblocks)
                        and qs < qa + 2
                        and qblocks[qs][2] <= k_cov
                    ):
                        states[qs] = emit_qblock_score(
                            qblocks[qs][0], qblocks[qs][1]
                        )
                        qs += 1
                        progress = True
                    if qa < qs and qblocks[qa][2] <= v_cov:
                        emit_qblock_av(states.pop(qa))
                        qa += 1
                        progress = True

            slice_starts = list(range(0, SVP, 512))
            v_pending = []
            k_cov = v_cov = 0
            for idx, s in enumerate(slice_starts):
                emit_qk_slice(s)
                k_cov = min(s + 512, SVP)
                try_emit(k_cov, v_cov)
                v_pending.append(s)
                if len(v_pending) == 2:
                    a, b = v_pending
                    if min(512, SVP - a) == min(512, SVP - b):
                        emit_v_pair(a, b)
                        v_pending = []
                    else:
                        emit_v_pair(a, None)
                        v_pending = [b]
                    v_cov = min((s + 512 if not v_pending else s), SVP)
                if idx == len(slice_starts) - 1:
                    for a in v_pending:
                        emit_v_pair(a, None)
                    v_pending = []
                    v_cov = SVP
                try_emit(k_cov, v_cov)
            assert qa == len(qblocks), (qa, qs)
    return nc


def _prepare(hidden_state, attention_masks, Wq, bq, Wk, bk, Wv, bv):
    """Host-side shard prep: sort each sequence into [valid | pad],
    pad both groups to shared multiples of 128, cast to bf16."""
    hs = np.asarray(hidden_state, dtype=np.float32)
    m = np.asarray(attention_masks)
    perms, nvs = [], []
    for b in range(B):
        mb = np.asarray(m[b]).astype(np.int64)
        perms.append(np.argsort(1 - mb, kind="stable"))
        nvs.append(int(mb.sum()))
    nps = [S - nv for nv in nvs]
    SV = max(128, -(-max(nvs) // P) * P)
    SPn = max(128, -(-max(nps) // P) * P)
    SVP = SV + SPn
    NSL = (SVP + 511) // 512
    WT = SVP - (NSL - 1) * 512

    wqk = np.ascontiguousarray(
        np.concatenate(
            [np.asarray(Wq, np.float32) / np.sqrt(H), np.asarray(Wk, np.float32)],
            axis=1,
        ).reshape(8, P, P).transpose(1, 0, 2)
    ).astype(BF)  # [p, c, m]
    wv = np.ascontiguousarray(
        np.asarray(Wv, np.float32).reshape(8, P, H).transpose(1, 0, 2)
    ).astype(BF)  # [p, c, m]
    # k takes NO bias on device: softmax is invariant to the per-query
    # constant it would add, and zero slot-pad k columns are required
    # for the kill-free masking (see module docstring).
    bqk = np.concatenate(
        [np.asarray(bq, np.float32) / np.sqrt(H), np.zeros(H, np.float32)]
    ).reshape(P, 1).astype(np.float32)

    # c01[j, 512+y] = 1.0 iff j <= y   (keep when q_rel - d >= j)
    y = np.arange(1024) - 512
    c01 = (np.arange(P)[:, None] <= y[None, :]).astype(BF)

    in_maps = []
    for b in range(B):
        nv, npd = nvs[b], nps[b]
        hs_sorted = np.zeros((NSL * 512, E), np.float32)
        hs_sorted[:npd] = hs[b][perms[b][nv:]]
        hs_sorted[SPn : SPn + nv] = hs[b][perms[b][:nv]]
        # full slices [128, NSF, 8, 512]:
        #   hsT[p, si, c, j] = hs_sorted[si*512+j, c*128+p]
        hsT = np.ascontiguousarray(
            hs_sorted[: (NSL - 1) * 512]
            .reshape(NSL - 1, 512, 8, P)
            .transpose(3, 0, 2, 1)
        ).astype(BF)
        # dense tail slice [128, 8, WT]
        hsTt = np.ascontiguousarray(
            hs_sorted[(NSL - 1) * 512 : (NSL - 1) * 512 + WT]
            .reshape(WT, 8, P)
            .transpose(2, 1, 0)
        ).astype(BF)
        in_maps.append(
            {
                "hsT": hsT,
                "hsTt": hsTt,
                "wqk": wqk,
                "wv": wv,
                "bqk": bqk,
                "c01": c01,
            }
        )
    return in_maps, perms, nvs, SV, SPn


def _finish_host(ot, perm, nv, SV, SPn, bv):
    """Normalize, unsort, restore bias for one batch element."""
    npd = S - nv
    den = ot[H].copy()
    # pad-part denominators carry one exp(0)=1 per slot-padded key
    den[:SPn] -= SPn - npd
    with np.errstate(divide="ignore", invalid="ignore", over="ignore"):
        dev = (ot[:H] / den).T
    out = np.empty((S, H), np.float32)
    out[perm[:nv]] = dev[SPn : SPn + nv]
    out[perm[nv:]] = dev[:npd]
    return out + bv


def _run(inputs: dict, trace: bool = False):
    from concourse import bass_utils

    in_maps, perms, nvs, SV, SPn = _prepare(**inputs)
    key = (SV, SPn)
    if key not in _NC_CACHE:
        _NC_CACHE[key] = build_nc(SV, SPn)
    nc = _NC_CACHE[key]

    res = bass_utils.run_bass_kernel_spmd(
        nc, in_maps, core_ids=list(range(8)), trace=trace
    )

    bv = np.asarray(inputs["bv"], np.float32)
    out = np.empty((B, S, H), np.float32)
    for b in range(B):
        out[b] = _finish_host(res.results[b]["outT"], perms[b], nvs[b], SV, SPn, bv)
    return out, res


def kernel(**inputs) -> np.ndarray:
    out, _ = _run(inputs, trace=False)
    return out
